# revision 29
# baseline (speedup 1.0000x reference)
"""Self-contained GATv2 node-classifier kernel for 8 Trainium2 NeuronCores.

Strategy: partition graphs (contiguous node ranges) across the 8 cores; each
core owns the edges whose dst it owns, sorted by dst into 128-edge tiles
grouped by 128-node dst blocks.  Per conv layer: per-core node transforms
(xl/xr/residual via PE), AllGather of the fp16 xl table, then a per-block edge
pipeline: one batched indirect-DMA gather of xl[src] and one of xr[dst]
(TB*128 descriptors each), GATv2 attention scores via a fold-tree dot with
att, exp written as duplicated pairs so the softmax-weighting multiply runs in
the DVE 2x fp16 mode, and a one-hot segment matmul (host-precomputed one-hot)
that accumulates softmax numerator and denominator in PSUM.  GraphNorm uses
host-precomputed per-graph one-hots for both stats and per-node expansion;
all compute in fp16 with f32 accumulation.
"""
import sys, os
for p in ('/opt/trn_rl_repo', '/root/.axon_site/_ro/trn_rl_repo'):
    if os.path.isdir(p) and p not in sys.path:
        sys.path.insert(0, p)
import re
import itertools
import numpy as np
import bass_rust
import concourse.bass as bass
import concourse.tile as tile
import concourse.mybir as mybir
from concourse.bass import IndirectOffsetOnAxis
from concourse.bass_utils import run_bass_kernel_spmd
from concourse.vector_clock import ScopedClock

WLIM = 1  # walrus-safe number of waits per instruction
_ws_counter = itertools.count()


def _split_waits_in_ordered(ordered):
    for bb_name, insts in ordered.items():
        new_list = []
        for inst in insts:
            si = inst.sync_info
            waits = list(si.on_wait) if (si is not None and si.on_wait) else []
            if len(waits) > WLIM:
                keep = waits[:WLIM]
                excess = waits[WLIM:]
                for w in excess:
                    ev = mybir.InstEventSemaphore(
                        name=f"WSPLIT-{next(_ws_counter)}",
                        ins=[], outs=[],
                        sync_info=bass_rust.SyncInfo(on_wait=[w], on_update=[]),
                    )
                    ev.engine = inst.engine
                    new_list.append(ev)
                inst.sync_info = bass_rust.SyncInfo(
                    on_wait=keep,
                    on_update=list(si.on_update) if si.on_update else [],
                )
            new_list.append(inst)
        ordered[bb_name] = new_list
    return ordered


def patch_tile(tile_mod):
    if getattr(tile_mod, "_walrus_wait_patched", False):
        return
    orig_postorder = tile_mod.postorder_instruction_blocks

    def patched_postorder(ordered, start_bb, postordered, *a, **k):
        _split_waits_in_ordered(ordered)
        return orig_postorder(ordered, start_bb, postordered, *a, **k)

    tile_mod.postorder_instruction_blocks = patched_postorder

    TileContext = tile_mod.TileContext

    def _drain_and_barrier(self, tick_clock, wait_clock):
        gc = tick_clock.global_clock
        vals = [int(s) for s in re.findall(r"-?\d+", str(gc))]
        for i, v in enumerate(vals):
            if v > 0:
                part = [0] * len(vals)
                part[i] = v
                nop_inst = self.nc.sync.nop(nofuse=True, hint="drain_split_wait")
                wait_clock.add_sem_waits(
                    nop_inst.ins, ScopedClock({None: bass_rust.VectorClock(part)})
                )
        self.nc.sync.drain()
        self.nc.all_engine_barrier()
        assert self.sems is not None
        popped = self.nc._tile_sem_poison_stack.pop()
        assert popped is self._sem_poison
        self.nc.clear_and_free_semaphores(list(self.sems.allocated().values()))
        self.nc.all_engine_barrier()

    TileContext._drain_and_barrier = _drain_and_barrier
    tile_mod._walrus_wait_patched = True

patch_tile(tile)

AF = mybir.ActivationFunctionType
OP = mybir.AluOpType
I32 = mybir.dt.int32
F32 = mybir.dt.float32

D = 176      # HC = H * HID
H = 11
HID = 16
DH2 = D + 2 * H  # 198: message cols + duplicated exp cols
IN = 128
OUT = 32
NCLS = 10
EPS = 1e-5
G = 64


# ---------------------------------------------------------------- host prep
def host_prep(inputs, n_cores=8, npdt=np.float16):
    x = np.asarray(inputs["x"], np.float32)
    ei = np.asarray(inputs["edge_index"]).astype(np.int64)
    batch = np.asarray(inputs["batch"]).astype(np.int64)
    N = x.shape[0]

    counts = np.bincount(batch, minlength=G)
    csum = np.concatenate([[0], np.cumsum(counts)])
    gb = [0]
    for c in range(1, n_cores):
        target = N * c / n_cores
        gi = int(np.argmin(np.abs(csum - target)))
        gi = max(gi, gb[-1] + 1)
        gi = min(gi, G - (n_cores - c))
        gb.append(gi)
    gb.append(G)
    node_start = np.array([csum[g] for g in gb], dtype=np.int64)
    ncs = np.diff(node_start)
    NB = int(np.ceil(ncs.max() / 128))
    NPAD = NB * 128
    GL = int(max(gb[c + 1] - gb[c] for c in range(n_cores))) + 1  # + pad graph
    NTOT = n_cores * NPAD

    owner = np.searchsorted(node_start[1:], np.arange(N), side="right")
    local = np.arange(N) - node_start[owner]
    padgid = (owner * NPAD + local).astype(np.int64)

    loops = np.arange(N, dtype=np.int64)
    src = np.concatenate([ei[0], loops])
    dst = np.concatenate([ei[1], loops])

    # per (core, block): edges split into local-src (gatherable from xlown
    # before the AllGather completes) and remote-src
    percore = []
    klocs = np.zeros((n_cores, NB), np.int64)
    ktots = np.zeros((n_cores, NB), np.int64)
    for c in range(n_cores):
        sel = owner[dst] == c
        s = src[sel]
        dl = dst[sel] - node_start[c]
        blk = dl >> 7
        order = np.argsort(blk, kind="stable")
        s = s[order]; dl = dl[order]; blk = blk[order]
        bounds = np.searchsorted(blk, np.arange(NB + 1))
        per = []
        for b in range(NB):
            sb = s[bounds[b]:bounds[b + 1]]
            db = dl[bounds[b]:bounds[b + 1]]
            isloc = owner[sb] == c
            per.append((sb, db, isloc))
            klocs[c, b] = int(isloc.sum())
            ktots[c, b] = len(sb)
        percore.append(per)
    # uniform-across-cores tile structure: nLoc_b local tiles first (every
    # block has >=128 local edges thanks to self-loops), then remote tiles
    nLocs = [max(1, int(min(np.ceil(klocs[:, b] / 128)))) for b in range(NB)]
    nRems = [max(1, int(max(np.ceil((ktots[:, b] - np.minimum(klocs[:, b], nLocs[b] * 128)) / 128))))
             for b in range(NB)]
    nTs = [nLocs[b] + nRems[b] for b in range(NB)]
    TB = max(nTs)
    ETILE = TB * 128

    # -------- shared (replicated) weight arrays
    f32 = lambda a: np.ascontiguousarray(np.asarray(a, np.float32))
    dt = lambda a: np.ascontiguousarray(np.asarray(a, np.float32).astype(npdt))
    Wcv = np.zeros((5, D, 528), np.float32)
    for i in range(4):
        Wcv[i, :, 0:176] = inputs["conv_Wl"][i]
        Wcv[i, :, 176:352] = inputs["conv_Wr"][i]
        Wcv[i, :, 352:528] = inputs["conv_Wres"][i]
    Wcv[4, :, 0:176] = inputs["c5_Wl"]
    Wcv[4, :, 176:352] = inputs["c5_Wr"]
    Wcv[4, :, 352:368] = inputs["c5_Wres"]

    attrep = np.zeros((5, 128, D), np.float32)
    for i in range(4):
        attrep[i] = np.tile(np.asarray(inputs["conv_att"][i]).reshape(1, D), (128, 1))
    attrep[4] = np.tile(np.asarray(inputs["c5_att"]).reshape(1, D), (128, 1))

    biasrep = np.zeros((5, 128, D), np.float32)
    for i in range(4):
        biasrep[i] = np.tile(np.asarray(inputs["conv_b"][i]).reshape(1, D), (128, 1))
    biasrep[4, :, 0:HID] = np.tile(np.asarray(inputs["c5_b"]).reshape(1, HID), (128, 1))

    shared = dict(
        Wpre=dt(inputs["W_pre"]),                     # [128, 176]
        Wcv=dt(Wcv),                                  # [5, 176, 528]
        attrep=dt(attrep),                            # [5, 128, 176]
        biasrep=f32(biasrep),                         # [5, 128, 176]
        bprerep=f32(np.tile(np.asarray(inputs["b_pre"]).reshape(1, D), (128, 1))),
        gnmsrep=f32(np.stack([np.tile(w.reshape(1, D), (GL, 1)) for w in np.asarray(inputs["gn_ms"])])),
        gnbT=f32(np.asarray(inputs["gn_b"]).reshape(5, D, 1)),
        ident=dt(np.eye(128, dtype=np.float32)),
        Wo1=dt(inputs["W_o1"]), Wo2=dt(inputs["W_o2"]), Wcls=dt(inputs["W_cls"]),
        bo1T=f32(np.asarray(inputs["b_o1"]).reshape(HID, 1)),
        bo2T=f32(np.asarray(inputs["b_o2"]).reshape(OUT, 1)),
        bclsT=f32(np.asarray(inputs["b_cls"]).reshape(NCLS, 1)),
    )

    in_maps = []
    ar128 = np.arange(128)
    for c in range(n_cores):
        per = percore[c]
        eoffT = np.zeros((NB, 128, 2 * TB), np.int32)
        OHt = np.zeros((NB, 128, ETILE), npdt)
        OHtT = np.zeros((NB, 128, ETILE), npdt)
        for b in range(NB):
            sb_all, db_all, isloc = per[b]
            k = len(sb_all)
            if k == 0:
                continue
            nL = nLocs[b]
            # local edges that fit the local region; the rest spill to remote
            li_idx = np.where(isloc)[0]
            spill = li_idx[nL * 128:]
            keep_loc = li_idx[:nL * 128]
            rem_idx = np.concatenate([np.where(~isloc)[0], spill])
            sb = np.zeros(ETILE, np.int64)
            db = np.full(ETILE, 255, np.int64)
            dloc = np.zeros(ETILE, np.int64)
            kl = len(keep_loc)
            # local region: offsets are LOCAL xlown rows
            sb[:kl] = sb_all[keep_loc] - node_start[c]
            db[:kl] = db_all[keep_loc] - b * 128
            dloc[:kl] = db_all[keep_loc]
            # remote region: offsets are global padded xlfull rows
            kr = len(rem_idx)
            r0 = nL * 128
            sb[r0:r0 + kr] = padgid[sb_all[rem_idx]]
            db[r0:r0 + kr] = db_all[rem_idx] - b * 128
            dloc[r0:r0 + kr] = db_all[rem_idx]
            eoffT[b, :, 0:TB] = sb.reshape(TB, 128).T
            eoffT[b, :, TB:2 * TB] = dloc.reshape(TB, 128).T
            dbt = db.reshape(TB, 128)
            for t in range(TB):
                oh = (dbt[t][:, None] == ar128[None, :]).astype(npdt)
                OHt[b, :, t * 128:(t + 1) * 128] = oh
                OHtT[b, :, t * 128:(t + 1) * 128] = oh.T

        n0 = int(node_start[c])
        nreal = int(ncs[c])
        xT = np.zeros((128, NPAD), np.float32)
        xT[:, :nreal] = x[n0:n0 + nreal].T
        blpad = np.full(NPAD, GL - 1, np.int64)
        blpad[:nreal] = batch[n0:n0 + nreal] - gb[c]
        invcnt = np.zeros((GL, 1), np.float32)
        for gi in range(gb[c], gb[c + 1]):
            invcnt[gi - gb[c], 0] = 1.0 / max(counts[gi], 1)
        gcount = gb[c + 1] - gb[c]
        gnwrep = np.zeros((5, GL, D), np.float32)
        gnwrep[:, :gcount, :] = np.asarray(inputs["gn_w"], np.float32)[:, None, :]
        # graph one-hots: per-block [128, GL] stacked along cols, and
        # transposed [GL, NB*128] for the per-node expansion matmuls
        bl2 = blpad.reshape(NB, 128)
        ohgH = np.zeros((128, NB * GL), npdt)
        ohgTH = np.zeros((GL, NB * 128), npdt)
        for b in range(NB):
            oh = (bl2[b][:, None] == np.arange(GL)[None, :])
            ohgH[:, b * GL:(b + 1) * GL] = oh.astype(npdt)
            ohgTH[:, b * 128:(b + 1) * 128] = oh.T.astype(npdt)
        in_maps.append(dict(
            xT=xT.astype(npdt), invcnt=invcnt, gnwrep=gnwrep,
            eoffT=eoffT, OHt=OHt, OHtT=OHtT, ohgH=ohgH, ohgTH=ohgTH, **shared))

    meta = dict(NB=NB, TB=TB, GL=GL, NPAD=NPAD, NTOT=NTOT, n_cores=n_cores,
                node_start=node_start, ncs=ncs, npdt=npdt, nTs=nTs,
                nLocs=nLocs)
    return in_maps, meta


# ---------------------------------------------------------------- kernel
def build_kernel(meta, dbg=False, DBGLI=0, DBGGI=0):
    NB, TB, GL = meta["NB"], meta["TB"], meta["GL"]
    nTs, nLocs = meta["nTs"], meta["nLocs"]
    NPAD, NTOT, NC = meta["NPAD"], meta["NTOT"], meta["n_cores"]
    DT = mybir.dt.float16 if meta["npdt"] == np.float16 else (
        mybir.dt.bfloat16 if meta["npdt"] == np.bfloat16 else F32)

    nc = bass.Bass()
    inp = {}
    def I(name, shape, dtype):
        inp[name] = nc.dram_tensor(name, shape, dtype, kind="ExternalInput")
        return inp[name]

    I("xT", [128, NPAD], DT)
    I("invcnt", [GL, 1], F32)
    I("eoffT", [NB, 128, 2 * TB], I32)
    I("OHt", [NB, 128, TB * 128], DT)
    I("OHtT", [NB, 128, TB * 128], DT)
    I("ohgH", [128, NB * GL], DT)
    I("ohgTH", [GL, NB * 128], DT)
    I("Wpre", [128, D], DT)
    I("Wcv", [5, D, 528], DT)
    I("attrep", [5, 128, D], DT)
    I("biasrep", [5, 128, D], F32)
    I("bprerep", [128, D], F32)
    I("gnwrep", [5, GL, D], F32)
    I("gnmsrep", [5, GL, D], F32)
    I("gnbT", [5, D, 1], F32)
    I("ident", [128, 128], DT)
    I("Wo1", [HID, HID], DT)
    I("Wo2", [HID, OUT], DT)
    I("Wcls", [OUT, NCLS], DT)
    I("bo1T", [HID, 1], F32)
    I("bo2T", [OUT, 1], F32)
    I("bclsT", [NCLS, 1], F32)

    outT = nc.dram_tensor("outT", [NCLS, NPAD], F32, kind="ExternalOutput")

    with tile.TileContext(nc) as tc:
      with tc.tile_pool(name="dram", bufs=1, space="DRAM") as dram, \
           tc.tile_pool(name="const", bufs=1) as constp, \
           tc.tile_pool(name="persist", bufs=1) as persist, \
           tc.tile_pool(name="lconst", bufs=2) as lconst, \
           tc.tile_pool(name="work", bufs=2) as work, \
           tc.tile_pool(name="psum_stat", bufs=1, space="PSUM") as pstat, \
           tc.tile_pool(name="psum_nt", bufs=2, space="PSUM") as pnt, \
           tc.tile_pool(name="psum_ed", bufs=2, space="PSUM") as ped, \
           tc.tile_pool(name="psum_tr", bufs=2, space="PSUM") as ptr:

        xlown = dram.tile([NPAD, D], DT)
        xlfull = dram.tile([NTOT, D], DT)

        # ---- constants
        ident = constp.tile([128, 128], DT)
        nc.sync.dma_start(out=ident[:], in_=inp["ident"][:])
        invcnt = constp.tile([GL, 1], F32)
        nc.sync.dma_start(out=invcnt[:], in_=inp["invcnt"][:])
        Wpre = constp.tile([128, D], DT)
        nc.sync.dma_start(out=Wpre[:], in_=inp["Wpre"][:])
        bprerep = constp.tile([128, D], F32)
        nc.sync.dma_start(out=bprerep[:], in_=inp["bprerep"][:])
        ohgH = constp.tile([128, NB * GL], DT)
        nc.sync.dma_start(out=ohgH[:], in_=inp["ohgH"][:])
        ohgTH = constp.tile([GL, NB * 128], DT)
        nc.sync.dma_start(out=ohgTH[:], in_=inp["ohgTH"][:])
        Wo1 = constp.tile([HID, HID], DT)
        nc.sync.dma_start(out=Wo1[:], in_=inp["Wo1"][:])
        Wo2 = constp.tile([HID, OUT], DT)
        nc.sync.dma_start(out=Wo2[:], in_=inp["Wo2"][:])
        Wcls = constp.tile([OUT, NCLS], DT)
        nc.sync.dma_start(out=Wcls[:], in_=inp["Wcls"][:])
        bo1T = constp.tile([HID, 1], F32)
        nc.sync.dma_start(out=bo1T[:], in_=inp["bo1T"][:])
        bo2T = constp.tile([OUT, 1], F32)
        nc.sync.dma_start(out=bo2T[:], in_=inp["bo2T"][:])
        bclsT = constp.tile([NCLS, 1], F32)
        nc.sync.dma_start(out=bclsT[:], in_=inp["bclsT"][:])

        # ---- persistent per-block tensors
        hTa = [persist.tile([128, 128], DT, tag=f"hTa{b}", name=f"hTa{b}") for b in range(NB)]
        hTb = [persist.tile([48, 128], DT, tag=f"hTb{b}", name=f"hTb{b}") for b in range(NB)]
        preall = [persist.tile([128, D], DT, tag=f"pre{b}", name=f"pre{b}") for b in range(NB)]
        resall = [persist.tile([128, D], DT, tag=f"res{b}", name=f"res{b}") for b in range(NB)]
        res5all = [persist.tile([128, HID], DT, tag=f"res5{b}", name=f"res5{b}") for b in range(NB)]
        xrall = [persist.tile([128, D], DT, tag=f"xr{b}", name=f"xr{b}") for b in range(NB)]
        denSall = [persist.tile([128, 2 * H], DT, tag=f"dS{b}", name=f"dS{b}") for b in range(NB)]
        h5T = persist.tile([HID, NPAD], DT)

        stats_x = pstat.tile([GL, D], F32, space="PSUM")
        stats_sq = pstat.tile([GL, D], F32, space="PSUM")

        # ---------------- helpers
        def gn_passA(b, pre):
            """pre tile already written; accumulate per-graph sums of x and x^2."""
            sq = work.tile([128, D], DT, tag="sq")
            nc.gpsimd.tensor_tensor(out=sq[:], in0=pre[:], in1=pre[:], op=OP.mult)
            ohg = ohgH[:, b * GL:(b + 1) * GL]
            nc.tensor.matmul(out=stats_x[:], lhsT=ohg, rhs=pre[:],
                             start=(b == 0), stop=(b == NB - 1))
            nc.tensor.matmul(out=stats_sq[:], lhsT=ohg, rhs=sq[:],
                             start=(b == 0), stop=(b == NB - 1))

        def gn_passB(gi, gnw, gnms, gnb0, gnb1):
            # layer-level stats [GL, 176]
            mean = work.tile([GL, D], F32, tag="gmean")
            nc.vector.tensor_scalar(out=mean[:], in0=stats_x[:],
                                    scalar1=invcnt[:, :1], scalar2=None, op0=OP.mult)
            ex2 = work.tile([GL, D], F32, tag="gex2")
            nc.vector.tensor_scalar(out=ex2[:], in0=stats_sq[:],
                                    scalar1=invcnt[:, :1], scalar2=None, op0=OP.mult)
            meanms = work.tile([GL, D], F32, tag="gmeanms")
            nc.vector.tensor_tensor(out=meanms[:], in0=mean[:], in1=gnms[:], op=OP.mult)
            u = work.tile([GL, D], F32, tag="gu")
            nc.vector.scalar_tensor_tensor(out=u[:], in0=mean[:], scalar=2.0,
                                           in1=meanms[:], op0=OP.mult, op1=OP.subtract)
            a2 = work.tile([GL, D], F32, tag="ga2")
            nc.vector.tensor_tensor(out=a2[:], in0=meanms[:], in1=u[:], op=OP.mult)
            var = work.tile([GL, D], F32, tag="gvar")
            nc.vector.scalar_tensor_tensor(out=var[:], in0=ex2[:], scalar=EPS,
                                           in1=a2[:], op0=OP.add, op1=OP.subtract)
            std = work.tile([GL, D], F32, tag="gstd")
            nc.scalar.activation(out=std[:], in_=var[:], func=AF.Sqrt)
            rstd = work.tile([GL, D], F32, tag="grstd")
            nc.vector.reciprocal(out=rstd[:], in_=std[:])
            f = work.tile([GL, D], DT, tag="gf")
            nc.vector.tensor_tensor(out=f[:], in0=gnw[:], in1=rstd[:], op=OP.mult)
            meanmsDT = work.tile([GL, D], DT, tag="gmmdt")
            nc.scalar.copy(out=meanmsDT[:], in_=meanms[:])

            for b in range(NB):
                ohgt = ohgTH[:, b * 128:(b + 1) * 128]
                ps_me = ped.tile([128, D], F32, space="PSUM", tag="ed", name="ps_me")
                nc.tensor.matmul(out=ps_me[:], lhsT=ohgt, rhs=meanmsDT[:],
                                 start=True, stop=True)
                nc.vector.tensor_tensor(out=preall[b][:], in0=preall[b][:],
                                        in1=ps_me[:], op=OP.subtract)
                ps_f = ped.tile([128, D], F32, space="PSUM", tag="ed", name="ps_f")
                nc.tensor.matmul(out=ps_f[:], lhsT=ohgt, rhs=f[:], start=True, stop=True)
                nc.vector.tensor_tensor(out=preall[b][:], in0=preall[b][:],
                                        in1=ps_f[:], op=OP.mult)
                ps_ta = ptr.tile([128, 128], DT, space="PSUM", tag="tr", name="ps_trA", bufs=1)
                nc.tensor.transpose(out=ps_ta[:], in_=preall[b][:, 0:128], identity=ident[:])
                nc.scalar.activation(out=hTa[b][:], in_=ps_ta[:], func=AF.Relu, bias=gnb0[:, :1])
                ps_tb = ptr.tile([48, 128], DT, space="PSUM", tag="tr", name="ps_trB", bufs=1)
                nc.tensor.transpose(out=ps_tb[:], in_=preall[b][:, 128:D], identity=ident[:])
                nc.scalar.activation(out=hTb[b][:], in_=ps_tb[:], func=AF.Relu, bias=gnb1[:, :1])

        # ================= layer 0 (pre-transform + gn + relu)
        for b in range(NB):
            xTb = work.tile([128, 128], DT, tag="xTb", bufs=3)
            nc.sync.dma_start(out=xTb[:], in_=inp["xT"][:, b * 128:(b + 1) * 128])
            ps = ped.tile([128, D], F32, space="PSUM", tag="ed", name="ps_h0")
            nc.tensor.matmul(out=ps[:], lhsT=xTb[:], rhs=Wpre[:],
                             start=True, stop=True)
            nc.vector.tensor_tensor(out=preall[b][:], in0=ps[:], in1=bprerep[:], op=OP.add)
            gn_passA(b, preall[b])
        gnw0 = lconst.tile([GL, D], F32, tag="gnw")
        nc.sync.dma_start(out=gnw0[:], in_=inp["gnwrep"][0])
        gnms0 = lconst.tile([GL, D], F32, tag="gnms")
        nc.sync.dma_start(out=gnms0[:], in_=inp["gnmsrep"][0])
        gnb0a = lconst.tile([128, 1], F32, tag="gnb0")
        nc.sync.dma_start(out=gnb0a[:], in_=inp["gnbT"][0, 0:128])
        gnb0b = lconst.tile([48, 1], F32, tag="gnb1")
        nc.sync.dma_start(out=gnb0b[:], in_=inp["gnbT"][0, 128:D])
        gn_passB(0, gnw0, gnms0, gnb0a, gnb0b)

        # ================= conv layers 1..5
        for li in range(5):
            Wcv0 = lconst.tile([128, 528], DT, tag="Wcv0")
            nc.sync.dma_start(out=Wcv0[:], in_=inp["Wcv"][li, 0:128])
            Wcv1 = lconst.tile([48, 528], DT, tag="Wcv1")
            nc.sync.dma_start(out=Wcv1[:], in_=inp["Wcv"][li, 128:D])
            attrep = lconst.tile([128, D], DT, tag="attrep")
            nc.sync.dma_start(out=attrep[:], in_=inp["attrep"][li])
            biasrep = lconst.tile([128, D], F32, tag="biasrep")
            nc.sync.dma_start(out=biasrep[:], in_=inp["biasrep"][li])

            # ---- node transforms
            for b in range(NB):
                ps_a = pnt.tile([128, 352], F32, space="PSUM", tag="ntA", name="ps_a", bufs=1)
                nc.tensor.matmul(out=ps_a[:], lhsT=hTa[b][:], rhs=Wcv0[:, 0:352],
                                 start=True, stop=False)
                nc.tensor.matmul(out=ps_a[:], lhsT=hTb[b][:], rhs=Wcv1[:, 0:352],
                                 start=False, stop=True)
                ps_r = ped.tile([128, D], F32, space="PSUM", tag="ed", name="ps_r")
                nc.tensor.matmul(out=ps_r[:], lhsT=hTa[b][:], rhs=Wcv0[:, 352:528],
                                 start=True, stop=False)
                nc.tensor.matmul(out=ps_r[:], lhsT=hTb[b][:], rhs=Wcv1[:, 352:528],
                                 start=False, stop=True)
                xl_sb = work.tile([128, D], DT, tag="xlsb")
                nc.scalar.copy(out=xl_sb[:], in_=ps_a[:, 0:D])
                nc.sync.dma_start(out=xlown[b * 128:(b + 1) * 128, :], in_=xl_sb[:])
                nc.scalar.copy(out=xrall[b][:], in_=ps_a[:, D:2 * D])
                if li < 4:
                    nc.vector.tensor_tensor(out=resall[b][:], in0=ps_r[:],
                                            in1=biasrep[:], op=OP.add)
                else:
                    nc.vector.tensor_tensor(out=res5all[b][:], in0=ps_r[:, 0:HID],
                                            in1=biasrep[:, 0:HID], op=OP.add)

            # ---- AllGather xl
            nc.gpsimd.collective_compute(
                "AllGather", OP.bypass,
                replica_groups=[list(range(NC))],
                ins=[xlown[:].opt()], outs=[xlfull[:].opt()])

            # ---- edge pipeline over a tile range of one dst block
            def edge_tiles(b, t0, t1, table, attrep):
                span = t1 - t0
                eo = work.tile([128, TB], I32, tag="eo", bufs=4)
                nc.sync.dma_start(out=eo[:, 0:span], in_=inp["eoffT"][b, :, t0:t1])
                OHtb = work.tile([128, TB * 128], DT, tag="OHtb", bufs=3)
                nc.sync.dma_start(out=OHtb[:, 0:span * 128],
                                  in_=inp["OHt"][b, :, t0 * 128:t1 * 128])
                OHtTb = work.tile([128, TB * 128], DT, tag="OHtTb", bufs=2)
                nc.sync.dma_start(out=OHtTb[:, 0:span * 128],
                                  in_=inp["OHtT"][b, :, t0 * 128:t1 * 128])
                XS = work.tile([128, TB * D], DT, tag="XS", bufs=3)
                XD = work.tile([128, TB * D], DT, tag="XD", bufs=3)
                for t in range(span):
                    nc.gpsimd.indirect_dma_start(
                        out=XS[:, t * D:(t + 1) * D], out_offset=None,
                        in_=table[:],
                        in_offset=IndirectOffsetOnAxis(ap=eo[:, t:t + 1], axis=0))
                    # xr[dst] via one-hot broadcast on PE (dst rows are this
                    # block's own 128 rows); fused with the xl+xr edge sum
                    ps_xd = ptr.tile([128, D], F32, space="PSUM", tag="xd", name="ps_xd", bufs=1)
                    nc.tensor.matmul(out=ps_xd[:], lhsT=OHtTb[:, t * 128:(t + 1) * 128],
                                     rhs=xrall[b][:], start=True, stop=True)
                    nc.vector.tensor_tensor(out=XD[:, t * D:(t + 1) * D],
                                            in0=XS[:, t * D:(t + 1) * D],
                                            in1=ps_xd[:], op=OP.add)
                LL = work.tile([128, TB * D], DT, tag="LL")
                nc.scalar.activation(out=LL[:, 0:span * D], in_=XD[:, 0:span * D],
                                     func=AF.Prelu, alpha=0.2)
                AT = LL
                nc.vector.tensor_tensor(
                    out=AT[:, 0:span * D].rearrange("p (t d) -> p t d", t=span),
                    in0=LL[:, 0:span * D].rearrange("p (t d) -> p t d", t=span),
                    in1=attrep[:].rearrange("p (one d) -> p one d", one=1).to_broadcast([128, span, D]),
                    op=OP.mult)
                F1 = work.tile([128, TB * H * 8], DT, tag="F1")
                atv = AT[:, 0:span * D].rearrange("p (th c) -> p th c", c=16)
                nc.vector.tensor_tensor(
                    out=F1[:, 0:span * H * 8].rearrange("p (th c) -> p th c", c=8),
                    in0=atv[:, :, 0:8], in1=atv[:, :, 8:16], op=OP.add)
                F2 = work.tile([128, TB * H * 4], DT, tag="F2")
                f1v = F1[:, 0:span * H * 8].rearrange("p (th c) -> p th c", c=8)
                nc.vector.tensor_tensor(
                    out=F2[:, 0:span * H * 4].rearrange("p (th c) -> p th c", c=4),
                    in0=f1v[:, :, 0:4], in1=f1v[:, :, 4:8], op=OP.add)
                ALPH = work.tile([128, TB * H], F32, tag="ALPH")
                nc.vector.tensor_reduce(
                    out=ALPH[:, 0:span * H].rearrange("p (th one) -> p th one", one=1),
                    in_=F2[:, 0:span * H * 4].rearrange("p (th c) -> p th c", c=4),
                    axis=mybir.AxisListType.X, op=OP.add)
                E2 = work.tile([128, TB * 2 * H], DT, tag="E2", bufs=3)
                nc.scalar.activation(
                    out=E2[:, 0:span * 2 * H].rearrange("p (th j) -> p th j", j=2),
                    in_=ALPH[:, 0:span * H].rearrange("p (th one) -> p th one", one=1)
                        .to_broadcast([128, span * H, 2]),
                    func=AF.Exp)
                XSW = work.tile([128, TB * D], DT, tag="XSW", bufs=2)
                nc.vector.tensor_tensor(
                    out=XSW[:, 0:span * D].rearrange("p (th c2 j) -> p th c2 j", c2=8, j=2),
                    in0=XS[:, 0:span * D].rearrange("p (th c2 j) -> p th c2 j", c2=8, j=2),
                    in1=E2[:, 0:span * 2 * H].rearrange("p (th one j) -> p th one j", one=1, j=2)
                        .to_broadcast([128, span * H, 8, 2]),
                    op=OP.mult)
                ps_ed = ped.tile([128, D], F32, space="PSUM", tag="ed", name="ps_ed")
                ps_den = ped.tile([128, 2 * H], F32, space="PSUM", tag="den", name="ps_den", bufs=1)
                for t in range(span):
                    nc.tensor.matmul(out=ps_ed[:],
                                     lhsT=OHtb[:, t * 128:(t + 1) * 128],
                                     rhs=XSW[:, t * D:(t + 1) * D],
                                     start=(t == 0), stop=(t == span - 1))
                    nc.tensor.matmul(out=ps_den[:],
                                     lhsT=OHtb[:, t * 128:(t + 1) * 128],
                                     rhs=E2[:, t * 2 * H:(t + 1) * 2 * H],
                                     start=(t == 0), stop=(t == span - 1))
                return ps_ed, ps_den

            # ---- local pass: tiles whose sources live on this core gather
            # from xlown and run WHILE the AllGather is in flight; partial
            # num/den parked in SBUF (preall is dead in this window)
            for b in range(NB):
                ps_ed, ps_den = edge_tiles(b, 0, nLocs[b], xlown, attrep)
                nc.scalar.copy(out=preall[b][:], in_=ps_ed[:])
                nc.scalar.copy(out=denSall[b][:], in_=ps_den[:])

            # ---- remote pass (needs the AllGather) + combine
            for b in range(NB):
                ps_ed, ps_den = edge_tiles(b, nLocs[b], nTs[b], xlfull, attrep)
                den2 = work.tile([128, 2 * H], F32, tag="den2")
                nc.vector.scalar_tensor_tensor(out=den2[:], in0=ps_den[:], scalar=1e-16,
                                               in1=denSall[b][:], op0=OP.add, op1=OP.add)
                rec2 = work.tile([128, 2 * H], F32, tag="rec2")
                nc.vector.reciprocal(out=rec2[:], in_=den2[:])
                m0 = work.tile([128, D], F32, tag="m0")
                nc.vector.tensor_tensor(out=m0[:], in0=ps_ed[:], in1=preall[b][:], op=OP.add)
                msg = work.tile([128, D], F32, tag="msg")
                nc.vector.tensor_tensor(
                    out=msg[:].rearrange("p (h c2 j) -> p h c2 j", c2=8, j=2),
                    in0=m0[:].rearrange("p (h c2 j) -> p h c2 j", c2=8, j=2),
                    in1=rec2[:].rearrange("p (h one j) -> p h one j", one=1, j=2)
                        .to_broadcast([128, H, 8, 2]),
                    op=OP.mult)
                if li < 4:
                    nc.vector.tensor_tensor(out=preall[b][:], in0=msg[:],
                                            in1=resall[b][:], op=OP.add)
                    gn_passA(b, preall[b])
                else:
                    m16 = work.tile([128, HID], F32, tag="m16")
                    nc.vector.tensor_reduce(
                        out=m16[:],
                        in_=msg[:].rearrange("p (h c) -> p c h", h=H),
                        axis=mybir.AxisListType.X, op=OP.add)
                    pre5 = work.tile([128, HID], DT, tag="pre5")
                    nc.vector.scalar_tensor_tensor(
                        out=pre5[:], in0=m16[:], scalar=1.0 / H,
                        in1=res5all[b][:], op0=OP.mult, op1=OP.add)
                    ps_t5 = ptr.tile([HID, 128], DT, space="PSUM", tag="tr", name="ps_t5", bufs=1)
                    nc.tensor.transpose(out=ps_t5[:], in_=pre5[:], identity=ident[:])
                    nc.scalar.activation(out=h5T[:, b * 128:(b + 1) * 128],
                                         in_=ps_t5[:], func=AF.Relu)
            if li < 4:
                gnw = lconst.tile([GL, D], F32, tag="gnw")
                nc.sync.dma_start(out=gnw[:], in_=inp["gnwrep"][li + 1])
                gnms = lconst.tile([GL, D], F32, tag="gnms")
                nc.sync.dma_start(out=gnms[:], in_=inp["gnmsrep"][li + 1])
                gnba = lconst.tile([128, 1], F32, tag="gnb0")
                nc.sync.dma_start(out=gnba[:], in_=inp["gnbT"][li + 1, 0:128])
                gnbb = lconst.tile([48, 1], F32, tag="gnb1")
                nc.sync.dma_start(out=gnbb[:], in_=inp["gnbT"][li + 1, 128:D])
                gn_passB(li + 1, gnw, gnms, gnba, gnbb)

        # ================= MLP head (transposed layout, column chunks)
        CH = 512
        for c0 in range(0, NPAD, CH):
            c1 = min(c0 + CH, NPAD)
            w = c1 - c0
            ps1 = pnt.tile([HID, CH], F32, space="PSUM", tag="ntA", name="ps1", bufs=1)
            nc.tensor.matmul(out=ps1[:, :w], lhsT=Wo1[:], rhs=h5T[:, c0:c1],
                             start=True, stop=True)
            y1 = work.tile([HID, CH], DT, tag="y1")
            nc.scalar.activation(out=y1[:, :w], in_=ps1[:, :w], func=AF.Relu, bias=bo1T[:, :1])
            ps2 = pnt.tile([OUT, CH], F32, space="PSUM", tag="ntA", name="ps2", bufs=1)
            nc.tensor.matmul(out=ps2[:, :w], lhsT=Wo2[:], rhs=y1[:, :w],
                             start=True, stop=True)
            y2 = work.tile([OUT, CH], DT, tag="y2")
            nc.scalar.activation(out=y2[:, :w], in_=ps2[:, :w], func=AF.Relu, bias=bo2T[:, :1])
            ps3 = pnt.tile([NCLS, CH], F32, space="PSUM", tag="ntA", name="ps3", bufs=1)
            nc.tensor.matmul(out=ps3[:, :w], lhsT=Wcls[:], rhs=y2[:, :w],
                             start=True, stop=True)
            y3 = work.tile([NCLS, CH], F32, tag="y3")
            nc.scalar.activation(out=y3[:, :w], in_=ps3[:, :w], func=AF.Identity, bias=bclsT[:, :1])
            nc.sync.dma_start(out=outT[:, c0:c1], in_=y3[:, :w])

    return nc


# ---------------------------------------------------------------- runner
def run(inputs, n_cores=8, npdt=np.float16, dbg=False, trace=False, DBGLI=0, DBGGI=0):
    in_maps, meta = host_prep(inputs, n_cores=n_cores, npdt=npdt)
    nc = build_kernel(meta, dbg=dbg, DBGLI=DBGLI, DBGGI=DBGGI)
    res = run_bass_kernel_spmd(nc, in_maps, core_ids=list(range(n_cores)), trace=trace)
    node_start, ncs = meta["node_start"], meta["ncs"]
    N = int(node_start[-1])
    out = np.zeros((N, NCLS), np.float32)
    for c in range(n_cores):
        o = res.results[c]["outT"]
        out[node_start[c]:node_start[c] + ncs[c]] = o[:, :ncs[c]].T
    return out, res, meta


def kernel(**inputs):
    in_maps, meta = host_prep(inputs, n_cores=8, npdt=np.float16)
    nc = build_kernel(meta, dbg=False)
    res = run_bass_kernel_spmd(nc, in_maps, core_ids=list(range(8)))
    node_start, ncs = meta["node_start"], meta["ncs"]
    N = int(node_start[-1])
    out = np.zeros((N, NCLS), np.float32)
    for c in range(8):
        o = res.results[c]["outT"]
        k = int(ncs[c])
        out[int(node_start[c]):int(node_start[c]) + k] = o[:, :k].T
    return out

